# revision 1
# baseline (speedup 1.0000x reference)
"""Trainium2 Bass kernel for nn_MessagePassingBlock (GNN message passing).

Math (reference):
    h     = x @ W_msg                       # (N, D)
    msg   = (h[source] + rel_bias[edge_type]) * edge_weights[:, None]
    delta = segment_sum(msg, target, N)     # (N, D)
    out   = relu(x @ W_self + delta + b)

Distribution: target-sharded across 8 cores (no collectives). Core c owns
nodes [c*12544, (c+1)*12544); every edge lives on its target's core.

Per-core algorithm (all matmul-based, no per-edge transposes):
  For each 128-node target block b, accumulate over that block's edges
  (chunks of 128 edges, gathered via batched SWDGE dma_gather from a bf16
  mirror of x):
      sT[k, j] += sum_e xg[e, k] * w_e * [tgt_e == j]      (PE, bf16)
      CT[r, j] += sum_e [et_e == r] * w_e * [tgt_e == j]   (PE, bf16)
  then
      out_b = relu(sT^T @ W_msg + CT^T @ rel_bias + x_b @ W_self + b)
  The onehot operands are built with single fused DVE tensor_scalar ops.
  Edge weights are folded into the target-onehot; padding edges carry w=0
  so they contribute exactly zero (self-masking).

Gather: x is split into 4 row subtables (<=32767 rows, int16 indices);
one dma_gather instruction per (superblock of 14 blocks, subtable), spread
across the 4 SWDGE queues.
"""

import functools
import math

import numpy as np
import ml_dtypes

NUM_NODES = 100000
D = 128
NUM_REL = 8
N_CORES = 8
NODES_PER_CORE = 12544          # 98 blocks of 128
NBLK = NODES_PER_CORE // 128    # 98
SB_BLOCKS = 14                  # blocks per superblock
N_SB = NBLK // SB_BLOCKS        # 7
N_SUBT = 4
SUBT_ROWS = 25000               # rows per gather subtable

_kernel_cache = {}


def _build_and_compile(c_bt_key, nchunks_sbt, chunk_plan):
    """Build + compile the SPMD Bass kernel for a given static chunk layout.

    nchunks_sbt: [N_SB][N_SUBT] -> number of 128-edge chunks in that
        gather instruction.
    chunk_plan: [NBLK] -> list of (t, slot_in_sbt_tile, global_chunk_id)
        in processing order for that block.
    """
    import concourse.bacc as bacc
    import concourse.tile as tile
    import concourse.mybir as mybir
    from concourse.masks import make_identity

    NC_TOT = sum(sum(row) for row in nchunks_sbt)

    nc = bacc.Bacc(
        "TRN2",
        target_bir_lowering=False,
        debug=False,
        num_devices=N_CORES,
        num_swdge_queues=4,
    )
    f32 = mybir.dt.float32
    bf16 = mybir.dt.bfloat16
    i16 = mybir.dt.int16

    xbf = nc.dram_tensor("xbf", [NUM_NODES, D], bf16, kind="ExternalInput")
    x_shard = nc.dram_tensor("x_shard", [NODES_PER_CORE, D], f32, kind="ExternalInput")
    w_msg = nc.dram_tensor("w_msg", [D, D], f32, kind="ExternalInput")
    w_self = nc.dram_tensor("w_self", [D, D], f32, kind="ExternalInput")
    rel_bias = nc.dram_tensor("rel_bias", [NUM_REL, D], f32, kind="ExternalInput")
    bvec = nc.dram_tensor("bvec", [1, D], f32, kind="ExternalInput")
    # gather indices, already 16-partition-wrapped + replicated to 128
    n_idx_cols = sum(n * 128 // 16 for row in nchunks_sbt for n in row)
    gidx = nc.dram_tensor("gidx", [128, n_idx_cols], i16, kind="ExternalInput")
    ohw_meta = nc.dram_tensor("ohw_meta", [128, NC_TOT * 128], bf16, kind="ExternalInput")
    ohe_meta = nc.dram_tensor("ohe_meta", [128, NC_TOT * NUM_REL], bf16, kind="ExternalInput")
    out_d = nc.dram_tensor("out", [D, NODES_PER_CORE], f32, kind="ExternalOutput")

    with tile.TileContext(nc) as tc:
        with tc.tile_pool(name="const", bufs=1) as cpool, tc.tile_pool(
            name="meta", bufs=1
        ) as mpool, tc.tile_pool(name="gath", bufs=2) as gpool, tc.tile_pool(
            name="oh", bufs=2
        ) as ohpool, tc.tile_pool(name="blk", bufs=3) as bpool, tc.tile_pool(
            name="ps", bufs=2, space="PSUM"
        ) as pspool, tc.tile_pool(name="pso", bufs=2, space="PSUM") as psopool:
            # ---- constants ----
            ident = cpool.tile([128, 128], f32)
            make_identity(nc, ident[:])
            wmsg_f = cpool.tile([128, D], f32)
            nc.sync.dma_start(out=wmsg_f[:], in_=w_msg.ap())
            wmsg_b = cpool.tile([128, D], bf16)
            nc.vector.tensor_copy(out=wmsg_b[:], in_=wmsg_f[:])
            wself_f = cpool.tile([128, D], f32)
            nc.sync.dma_start(out=wself_f[:], in_=w_self.ap())
            rb_f = cpool.tile([NUM_REL, D], f32)
            nc.sync.dma_start(out=rb_f[:], in_=rel_bias.ap())
            rb_b = cpool.tile([NUM_REL, D], bf16)
            nc.vector.tensor_copy(out=rb_b[:], in_=rb_f[:])
            b_row = cpool.tile([1, D], f32)
            nc.sync.dma_start(out=b_row[:], in_=bvec.ap())
            ones1 = cpool.tile([1, 2 * D], f32)
            nc.vector.memset(ones1[:], 1.0)

            # ---- gather indices (one DMA) ----
            gidx_t = mpool.tile([128, n_idx_cols], i16)
            nc.sync.dma_start(out=gidx_t[:], in_=gidx.ap())

            # precompute static offsets
            idx_off = {}
            off = 0
            for sb in range(N_SB):
                for t in range(N_SUBT):
                    idx_off[(sb, t)] = off
                    off += nchunks_sbt[sb][t] * 128 // 16

            gmax = [max(nchunks_sbt[sb][t] for sb in range(N_SB)) for t in range(N_SUBT)]
            _starts = []
            for _g in range(0, NBLK, 7):
                _e = _g + 7
                _p0 = 0
                for _b in range(_g):
                    _p0 += len(chunk_plan[_b])
                _p1 = _p0
                for _b in range(_g, min(_e, NBLK)):
                    _p1 += len(chunk_plan[_b])
                _starts.append(_p1 - _p0)
            ghw_max = max(_starts)
            pos_of = {}
            _p = 0
            for _b in range(NBLK):
                pos_of[_b] = _p
                _p += len(chunk_plan[_b])

            PIECE = 16  # chunks per gather instruction (2048 idxs)
            swdge_i = 0
            for sb in range(N_SB):
                # ---- gather instructions for this superblock, in pieces ----
                gtiles = []
                for t in range(N_SUBT):
                    nck = nchunks_sbt[sb][t]
                    gt = gpool.tile([128, gmax[t] * 128], bf16, tag=f"g{t}")
                    base = t * SUBT_ROWS
                    rows = min(SUBT_ROWS, NUM_NODES - base)
                    io = idx_off[(sb, t)]
                    for p0 in range(0, nck, PIECE):
                        pk = min(PIECE, nck - p0)
                        n = pk * 128
                        nc.gpsimd.dma_gather(
                            out_ap=gt[:, p0 * 128 : (p0 + pk) * 128].rearrange(
                                "p (c r) -> p c r", r=128
                            ),
                            in_ap=xbf.ap()[base : base + rows, :],
                            idxs_ap=gidx_t[
                                :, io + p0 * 8 : io + (p0 + pk) * 8
                            ],
                            num_idxs=n,
                            num_idxs_reg=n,
                            elem_size=D,
                            single_packet=False,
                            queue_num=swdge_i % 4,
                        )
                        swdge_i += 1
                    gtiles.append(gt)

                for half in range(2):
                    g0 = sb * SB_BLOCKS + half * 7
                    p0 = pos_of[g0]
                    p1 = pos_of[g0 + 7] if g0 + 7 < NBLK else NC_TOT
                    nchv = p1 - p0
                    ghw = ohpool.tile([128, ghw_max * 128], bf16, tag="ghw")
                    nc.scalar.dma_start(
                        out=ghw[:, : nchv * 128],
                        in_=ohw_meta.ap()[:, p0 * 128 : p1 * 128],
                    )
                    ghe = ohpool.tile([128, ghw_max * NUM_REL], bf16, tag="ghe")
                    nc.scalar.dma_start(
                        out=ghe[:, : nchv * NUM_REL],
                        in_=ohe_meta.ap()[:, p0 * NUM_REL : p1 * NUM_REL],
                    )
                    x7 = bpool.tile([128, 7 * 128], f32, tag="x7")
                    nc.sync.dma_start(
                        out=x7[:],
                        in_=x_shard.ap()[g0 * 128 : (g0 + 7) * 128, :].rearrange(
                            "(c p) f -> p c f", p=128
                        ),
                    )
                    o7 = bpool.tile([128, 7 * 128], f32, tag="o7")
                    # per-block accumulation (sT / cT), then paired epilogues
                    sT_p = {}
                    cT_p = {}
                    for bi in range(7):
                        blk = g0 + bi
                        plan = chunk_plan[blk]
                        assert plan, f"block {blk} has no chunks"
                        nchunk = len(plan)
                        bpos = pos_of[blk] - p0
                        sT = pspool.tile([128, 128], f32, tag="sT")
                        cT = pspool.tile([NUM_REL, 128], f32, tag="cT")
                        sT_p[bi] = sT
                        cT_p[bi] = cT
                        for ci, (t, slot, gchunk) in enumerate(plan):
                            ohw = ghw[:, (bpos + ci) * 128 : (bpos + ci + 1) * 128]
                            xg = gtiles[t][:, slot * 128 : (slot + 1) * 128]
                            nc.tensor.matmul(
                                out=sT[:], lhsT=xg, rhs=ohw,
                                start=(ci == 0), stop=(ci == nchunk - 1),
                            )
                        for ci, (t, slot, gchunk) in enumerate(plan):
                            ohw = ghw[:, (bpos + ci) * 128 : (bpos + ci + 1) * 128]
                            ohe = ghe[
                                :,
                                (bpos + ci) * NUM_REL : (bpos + ci + 1) * NUM_REL,
                            ]
                            nc.tensor.matmul(
                                out=cT[:], lhsT=ohe, rhs=ohw,
                                start=(ci == 0), stop=(ci == nchunk - 1),
                            )
                        if bi % 2 == 1 or bi == 6:
                            lo = bi - 1 if bi % 2 == 1 else bi
                            nb = bi - lo + 1
                            w = nb * 128
                            sT_sb = bpool.tile([128, 256], bf16, tag="sTsb")
                            cT_sb = bpool.tile([NUM_REL, 256], bf16, tag="cTsb")
                            xT_sb = bpool.tile([128, 256], f32, tag="xTsb")
                            for k2 in range(nb):
                                b2 = lo + k2
                                nc.vector.tensor_copy(
                                    out=sT_sb[:, k2 * 128 : (k2 + 1) * 128],
                                    in_=sT_p[b2][:],
                                )
                                nc.vector.tensor_copy(
                                    out=cT_sb[:, k2 * 128 : (k2 + 1) * 128],
                                    in_=cT_p[b2][:],
                                )
                                xT_ps = psopool.tile([128, 128], f32, tag="xT")
                                nc.tensor.transpose(
                                    out=xT_ps[:],
                                    in_=x7[:, b2 * 128 : (b2 + 1) * 128],
                                    identity=ident[:],
                                )
                                nc.vector.tensor_copy(
                                    out=xT_sb[:, k2 * 128 : (k2 + 1) * 128],
                                    in_=xT_ps[:],
                                )
                            accT = psopool.tile([128, 256], f32, tag="accT")
                            nc.tensor.matmul(
                                out=accT[:, :w], lhsT=wmsg_b[:], rhs=sT_sb[:, :w],
                                start=True, stop=False,
                            )
                            nc.tensor.matmul(
                                out=accT[:, :w], lhsT=rb_b[:], rhs=cT_sb[:, :w],
                                start=False, stop=False,
                            )
                            nc.tensor.matmul(
                                out=accT[:, :w], lhsT=wself_f[:], rhs=xT_sb[:, :w],
                                start=False, stop=False,
                            )
                            nc.tensor.matmul(
                                out=accT[:, :w], lhsT=b_row[:], rhs=ones1[:, :w],
                                start=False, stop=True,
                            )
                            nc.scalar.activation(
                                out=o7[:, lo * 128 : lo * 128 + w],
                                in_=accT[:, :w],
                                func=mybir.ActivationFunctionType.Relu,
                            )
                    nc.sync.dma_start(
                        out=out_d.ap()[:, g0 * 128 : (g0 + 7) * 128],
                        in_=o7[:],
                    )

    nc.compile()
    return nc


def _prep(inputs):
    """Host-side sharding/layout. Returns (in_maps, static_key, layout)."""
    x = np.ascontiguousarray(np.asarray(inputs["x"], dtype=np.float32))
    source = np.asarray(inputs["source"]).astype(np.int64)
    target = np.asarray(inputs["target"]).astype(np.int64)
    edge_type = np.asarray(inputs["edge_type"]).astype(np.int64)
    ew = np.asarray(inputs["edge_weights"], dtype=np.float32)
    w_msg = np.ascontiguousarray(np.asarray(inputs["W_msg"], dtype=np.float32))
    rel_bias = np.ascontiguousarray(np.asarray(inputs["rel_bias"], dtype=np.float32))
    w_self = np.ascontiguousarray(np.asarray(inputs["W_self"], dtype=np.float32))
    b = np.asarray(inputs["b"], dtype=np.float32).reshape(1, D)

    n = x.shape[0]
    assert n == NUM_NODES

    xbf = x.astype(ml_dtypes.bfloat16)

    core = target // NODES_PER_CORE
    tgt_local = target - core * NODES_PER_CORE
    blk = tgt_local >> 7
    tgt_in_blk = tgt_local & 127
    subt = source // SUBT_ROWS
    src_local = source - subt * SUBT_ROWS

    # per (core, blk, subtable) edge index lists
    # order edges by (core, blk, subt) with a stable sort
    key = ((core * NBLK + blk) * N_SUBT + subt).astype(np.int64)
    order = np.argsort(key, kind="stable")
    key_s = key[order]
    # group boundaries
    uniq, starts = np.unique(key_s, return_index=True)
    counts = np.diff(np.append(starts, key_s.shape[0]))

    cnt = np.zeros((N_CORES, NBLK, N_SUBT), dtype=np.int64)
    ci = uniq // (NBLK * N_SUBT)
    bi = (uniq // N_SUBT) % NBLK
    ti = uniq % N_SUBT
    cnt[ci, bi, ti] = counts

    # static chunk capacity per (blk, subtable): max over cores
    c_bt = np.ceil(cnt.max(axis=0) / 128).astype(np.int64)  # (NBLK, N_SUBT)
    # ensure every block has at least one chunk
    empty = c_bt.sum(axis=1) == 0
    c_bt[empty, 0] = 1

    nchunks_sbt = [
        [int(c_bt[sb * SB_BLOCKS : (sb + 1) * SB_BLOCKS, t].sum()) for t in range(N_SUBT)]
        for sb in range(N_SB)
    ]
    NC_TOT = int(c_bt.sum())

    # global chunk ids: order is (sb, t, blk-within-sb, chunk)
    gchunk_of = np.zeros((NBLK, N_SUBT), dtype=np.int64)  # first chunk id
    slot_of = np.zeros((NBLK, N_SUBT), dtype=np.int64)    # first slot in (sb,t) tile
    g = 0
    for sb in range(N_SB):
        for t in range(N_SUBT):
            s = 0
            for bi2 in range(SB_BLOCKS):
                bb = sb * SB_BLOCKS + bi2
                gchunk_of[bb, t] = g
                slot_of[bb, t] = s
                g += int(c_bt[bb, t])
                s += int(c_bt[bb, t])
    assert g == NC_TOT

    chunk_plan = []
    for bb in range(NBLK):
        plan = []
        for t in range(N_SUBT):
            for c in range(int(c_bt[bb, t])):
                plan.append((t, int(slot_of[bb, t] + c), int(gchunk_of[bb, t] + c)))
        chunk_plan.append(plan)

    # position of each block's chunk run in the (block-major) onehot layout
    pos_of_blk = np.zeros(NBLK, dtype=np.int64)
    p = 0
    for bb in range(NBLK):
        pos_of_blk[bb] = p
        p += len(chunk_plan[bb])
    # gchunk -> block-major position
    pos_of_gchunk = np.zeros(NC_TOT, dtype=np.int64)
    for bb in range(NBLK):
        for i, (_t, _s, g2) in enumerate(chunk_plan[bb]):
            pos_of_gchunk[g2] = pos_of_blk[bb] + i

    n_idx_cols = sum(nc_ * 128 // 16 for row in nchunks_sbt for nc_ in row)

    # build per-core tensors
    in_maps = []
    # offsets of edge groups in the sorted edge array, per core
    start_of = {}
    for u, s0, c0 in zip(uniq, starts, counts):
        start_of[int(u)] = (int(s0), int(c0))

    for c in range(N_CORES):
        gidx = np.zeros((128, n_idx_cols), dtype=np.int16)
        ohw_m = np.zeros((128, NC_TOT * 128), dtype=ml_dtypes.bfloat16)
        ohe_m = np.zeros((128, NC_TOT * NUM_REL), dtype=ml_dtypes.bfloat16)

        icol = 0
        for sb in range(N_SB):
            for t in range(N_SUBT):
                nck = nchunks_sbt[sb][t]
                if nck == 0:
                    continue
                nslots = nck * 128
                idxs = np.zeros(nslots, dtype=np.int16)
                for bi2 in range(SB_BLOCKS):
                    bb = sb * SB_BLOCKS + bi2
                    k = (c * NBLK + bb) * N_SUBT + t
                    s0, n_e = start_of.get(k, (0, 0))
                    sl0 = int(slot_of[bb, t]) * 128 - int(slot_of[sb * SB_BLOCKS, t]) * 128
                    g0 = int(gchunk_of[bb, t])
                    if n_e:
                        eids = order[s0 : s0 + n_e]
                        idxs[sl0 : sl0 + n_e] = src_local[eids].astype(np.int16)
                        # meta: chunk-major [128 partitions]
                        for cc in range(int(c_bt[bb, t])):
                            lo = cc * 128
                            hi = min(n_e, lo + 128)
                            if hi <= lo:
                                break
                            ecol = eids[lo:hi]
                            gc = g0 + cc
                            npart = hi - lo
                            pos = int(pos_of_gchunk[gc])
                            parts = np.arange(npart)
                            ohw_m[parts, pos * 128 + tgt_in_blk[ecol]] = ew[
                                ecol
                            ].astype(ml_dtypes.bfloat16)
                            ohe_m[parts, pos * NUM_REL + edge_type[ecol]] = 1.0
                # wrap idxs: element j -> partition j%16, col j//16; replicate x8
                wrapped = idxs.reshape(nslots // 16, 16).T  # (16, nslots/16)
                gidx[:, icol : icol + nslots // 16] = np.tile(wrapped, (8, 1))
                icol += nslots // 16
        assert icol == n_idx_cols

        xs = np.zeros((NODES_PER_CORE, D), dtype=np.float32)
        lo = c * NODES_PER_CORE
        hi = min(lo + NODES_PER_CORE, NUM_NODES)
        xs[: hi - lo] = x[lo:hi]

        in_maps.append(
            {
                "xbf": xbf,
                "x_shard": xs,
                "w_msg": w_msg,
                "w_self": w_self,
                "rel_bias": rel_bias,
                "bvec": b,
                "gidx": gidx,
                "ohw_meta": ohw_m,
                "ohe_meta": ohe_m,
            }
        )

    static_key = tuple(c_bt.flatten().tolist())
    return in_maps, static_key, (nchunks_sbt, chunk_plan)


def kernel(**inputs) -> np.ndarray:
    from concourse import bass_utils

    in_maps, static_key, (nchunks_sbt, chunk_plan) = _prep(inputs)

    nc = _kernel_cache.get(static_key)
    if nc is None:
        nc = _build_and_compile(static_key, nchunks_sbt, chunk_plan)
        _kernel_cache[static_key] = nc

    res = bass_utils.run_bass_kernel_spmd(
        nc, in_maps, core_ids=list(range(N_CORES))
    )
    parts = [res.results[c]["out"].T for c in range(N_CORES)]
    full = np.concatenate(parts, axis=0)[:NUM_NODES]
    return np.ascontiguousarray(full, dtype=np.float32)



# revision 8
# speedup vs baseline: 1.2186x; 1.2186x over previous
"""Trainium2 Bass kernel for nn_MessagePassingBlock (GNN message passing).

Math (reference):
    h     = x @ W_msg                       # (N, D)
    msg   = (h[source] + rel_bias[edge_type]) * edge_weights[:, None]
    delta = segment_sum(msg, target, N)     # (N, D)
    out   = relu(x @ W_self + delta + b)

Distribution: target-sharded across 8 cores (no collectives). Core c owns
nodes [c*12544, (c+1)*12544); every edge lives on its target's core.

Per-core algorithm (v2):
  For each 128-node target block b, accumulate over that block's edge
  chunks (128 edges each, gathered via SWDGE dma_gather from a bf16
  mirror of x):
      sT[k, j] += sum_e xg[e, k] * ohw[e, j]          (PE, bf16)
  where ohw[e, j] = w_e * [tgt_e == j] is built ON-CHIP by DVE from
  compact per-edge metadata (tgt_in_blk, w) via batched iota-compare:
      eq  = (iota_rep == tgt_meta_bcast)      one tensor_tensor per (sb,t)
      ohw = eq * ew_meta_bcast                one tensor_tensor per (sb,t)
  The rel_bias term uses host-precomputed per-node weighted relation
  counts cnt_w[r, j] (tiny: 8 x 12544 bf16), so no per-chunk cT matmuls.
  Epilogue per 512-col segment (4 blocks):
      acc[d, j] = W_msg^T @ sT_seg + rel_bias^T @ cnt_seg + W_self^T @ xT_seg
      out[d, j] = relu(acc + b)               (ACT, bias folded in)
  x is pre-transposed on the host (xT_shard [D, nodes]) so no PE
  transposes are needed anywhere.

Gather: x is split into 4 row subtables (<=32767 rows, int16 indices);
ONE dma_gather instruction per (superblock of 14 blocks, subtable),
spread across the 4 SWDGE queues.
"""

import numpy as np
import ml_dtypes

NUM_NODES = 100000
D = 128
NUM_REL = 8
N_CORES = 8
NODES_PER_CORE = 12544          # 98 blocks of 128
NBLK = NODES_PER_CORE // 128    # 98
SB_BLOCKS = 14                  # blocks per superblock
N_SB = NBLK // SB_BLOCKS        # 7
N_SUBT = 4
SUBT_ROWS = 25000               # rows per gather subtable

_kernel_cache = {}


def _build_and_compile(c_bt_key, nchunks_sbt, chunk_plan, slotbase_sbt):
    """Build + compile the SPMD Bass kernel for a given static chunk layout.

    nchunks_sbt: [N_SB][N_SUBT] -> number of 128-edge chunks in that
        gather instruction.
    chunk_plan: [NBLK] -> list of (t, slot_in_sbt_tile, global_chunk_id)
        in processing order for that block.
    slotbase_sbt: [N_SB][N_SUBT] -> first global chunk id of that tile.
    """
    import concourse.bacc as bacc
    import concourse.tile as tile
    import concourse.mybir as mybir

    NC_TOT = sum(sum(row) for row in nchunks_sbt)

    nc = bacc.Bacc(
        "TRN2",
        target_bir_lowering=False,
        debug=False,
        num_devices=N_CORES,
        num_swdge_queues=4,
    )
    f32 = mybir.dt.float32
    bf16 = mybir.dt.bfloat16
    i16 = mybir.dt.int16

    xbf = nc.dram_tensor("xbf", [NUM_NODES, D], bf16, kind="ExternalInput")
    xT_shard = nc.dram_tensor("xT_shard", [D, NODES_PER_CORE], bf16, kind="ExternalInput")
    w_msg_b = nc.dram_tensor("w_msg_b", [D, D], bf16, kind="ExternalInput")
    w_self_b = nc.dram_tensor("w_self_b", [D, D], bf16, kind="ExternalInput")
    rb_b = nc.dram_tensor("rb_b", [NUM_REL, D], bf16, kind="ExternalInput")
    b_col = nc.dram_tensor("b_col", [D, 1], f32, kind="ExternalInput")
    iota_d = nc.dram_tensor("iota_d", [128, 128], bf16, kind="ExternalInput")
    cnt_w = nc.dram_tensor("cnt_w", [NUM_REL, NODES_PER_CORE], bf16, kind="ExternalInput")
    # gather indices, 16-partition-wrapped + replicated to 128
    n_idx_cols = NC_TOT * 8
    gidx = nc.dram_tensor("gidx", [128, n_idx_cols], i16, kind="ExternalInput")
    tgt_meta = nc.dram_tensor("tgt_meta", [128, NC_TOT], bf16, kind="ExternalInput")
    ew_meta = nc.dram_tensor("ew_meta", [128, NC_TOT], bf16, kind="ExternalInput")
    out_d = nc.dram_tensor("out", [D, NODES_PER_CORE], f32, kind="ExternalOutput")

    gmax = [max(nchunks_sbt[sb][t] for sb in range(N_SB)) for t in range(N_SUBT)]

    with tile.TileContext(nc) as tc:
        with tc.tile_pool(name="const", bufs=1) as cpool, tc.tile_pool(
            name="gath", bufs=2
        ) as gpool, tc.tile_pool(name="oh", bufs=2) as ohpool, tc.tile_pool(
            name="blk", bufs=2
        ) as bpool, tc.tile_pool(name="seg", bufs=3) as spool, tc.tile_pool(
            name="ps", bufs=5, space="PSUM"
        ) as pspool, tc.tile_pool(name="pso", bufs=2, space="PSUM") as psopool:
            # ---- constants (one-time loads) ----
            wmsg_t = cpool.tile([128, D], bf16)
            nc.sync.dma_start(out=wmsg_t[:], in_=w_msg_b.ap())
            wself_t = cpool.tile([128, D], bf16)
            nc.sync.dma_start(out=wself_t[:], in_=w_self_b.ap())
            rb_t = cpool.tile([NUM_REL, D], bf16)
            nc.sync.dma_start(out=rb_t[:], in_=rb_b.ap())
            bcol_t = cpool.tile([D, 1], f32)
            nc.sync.dma_start(out=bcol_t[:], in_=b_col.ap())
            iota_t = cpool.tile([128, 128], bf16)
            nc.sync.dma_start(out=iota_t[:], in_=iota_d.ap())
            gidx_t = cpool.tile([128, n_idx_cols], i16)
            nc.sync.dma_start(out=gidx_t[:], in_=gidx.ap())
            tgt_t = cpool.tile([128, NC_TOT], bf16)
            nc.sync.dma_start(out=tgt_t[:], in_=tgt_meta.ap())
            ew_t = cpool.tile([128, NC_TOT], bf16)
            nc.sync.dma_start(out=ew_t[:], in_=ew_meta.ap())

            # static gather-index column offsets
            idx_off = {}
            off = 0
            for sb in range(N_SB):
                for t in range(N_SUBT):
                    idx_off[(sb, t)] = off
                    off += nchunks_sbt[sb][t] * 8
            assert off == n_idx_cols

            swdge_i = 0
            for sb in range(N_SB):
                g0 = sb * SB_BLOCKS
                # ---- per-sb streamed inputs ----
                xT_sb = bpool.tile([128, SB_BLOCKS * 128], bf16, tag="xT")
                nc.scalar.dma_start(
                    out=xT_sb[:],
                    in_=xT_shard.ap()[:, g0 * 128 : (g0 + SB_BLOCKS) * 128],
                )
                cnt_sb = bpool.tile([NUM_REL, SB_BLOCKS * 128], bf16, tag="cnt")
                nc.scalar.dma_start(
                    out=cnt_sb[:],
                    in_=cnt_w.ap()[:, g0 * 128 : (g0 + SB_BLOCKS) * 128],
                )

                # ---- gather + on-chip onehot build per subtable ----
                gtiles = []
                ohtiles = []
                for t in range(N_SUBT):
                    nck = nchunks_sbt[sb][t]
                    gt = gpool.tile([128, gmax[t] * 128], bf16, tag=f"g{t}")
                    oht = ohpool.tile([128, gmax[t] * 128], bf16, tag=f"oh{t}")
                    if nck:
                        base = t * SUBT_ROWS
                        rows = min(SUBT_ROWS, NUM_NODES - base)
                        io = idx_off[(sb, t)]
                        n = nck * 128
                        nc.gpsimd.dma_gather(
                            out_ap=gt[:, : n].rearrange("p (c r) -> p c r", r=128),
                            in_ap=xbf.ap()[base : base + rows, :],
                            idxs_ap=gidx_t[:, io : io + nck * 8],
                            num_idxs=n,
                            num_idxs_reg=n,
                            elem_size=D,
                            single_packet=False,
                            queue_num=swdge_i % 4,
                        )
                        swdge_i += 1
                        # onehot: ohw[e, c*128+j] = (iota[j]==tgt[e,c]) * ew[e,c]
                        c0 = slotbase_sbt[sb][t]
                        oh3 = oht[:, : n].rearrange("p (c r) -> p c r", r=128)
                        iota3 = iota_t[:, :].rearrange(
                            "p (a r) -> p a r", a=1
                        ).to_broadcast([128, nck, 128])
                        tgt3 = tgt_t[:, c0 : c0 + nck].rearrange(
                            "p (c a) -> p c a", a=1
                        ).to_broadcast([128, nck, 128])
                        ew3 = ew_t[:, c0 : c0 + nck].rearrange(
                            "p (c a) -> p c a", a=1
                        ).to_broadcast([128, nck, 128])
                        nc.vector.tensor_tensor(
                            out=oh3, in0=iota3, in1=tgt3,
                            op=mybir.AluOpType.is_equal,
                        )
                        nc.vector.tensor_tensor(
                            out=oh3, in0=oh3, in1=ew3,
                            op=mybir.AluOpType.mult,
                        )
                    gtiles.append(gt)
                    ohtiles.append(oht)

                # ---- per-block chunk matmuls (accumulate sT in PSUM) ----
                # 4 blocks share one bank-sized PSUM tile [128, 512]
                seg_ps = {}
                for bi in range(SB_BLOCKS):
                    blk = g0 + bi
                    plan = chunk_plan[blk]
                    assert plan, f"block {blk} has no chunks"
                    nchunk = len(plan)
                    if bi % 4 == 0:
                        sT_bank = pspool.tile([128, 512], f32, tag="sT")
                        seg_ps[bi // 4] = sT_bank
                    sT = seg_ps[bi // 4][:, (bi % 4) * 128 : (bi % 4 + 1) * 128]
                    for ci, (t, slot, _gchunk) in enumerate(plan):
                        xg = gtiles[t][:, slot * 128 : (slot + 1) * 128]
                        ohw = ohtiles[t][:, slot * 128 : (slot + 1) * 128]
                        nc.tensor.matmul(
                            out=sT, lhsT=xg, rhs=ohw,
                            start=(ci == 0), stop=(ci == nchunk - 1),
                        )

                # ---- epilogue in 512-wide segments (4 blocks each) ----
                o14 = spool.tile([128, SB_BLOCKS * 128], f32, tag="o14")
                seg_starts = list(range(0, SB_BLOCKS, 4))  # 0,4,8,12
                for s0 in seg_starts:
                    nb = min(4, SB_BLOCKS - s0)
                    w = nb * 128
                    sT_sb = spool.tile([128, 512], bf16, tag="sTsb")
                    # PSUM -> SBUF cast copy on the scalar engine
                    nc.scalar.activation(
                        out=sT_sb[:, :w],
                        in_=seg_ps[s0 // 4][:, :w],
                        func=mybir.ActivationFunctionType.Copy,
                    )
                    accT = psopool.tile([128, 512], f32, tag="accT")
                    nc.tensor.matmul(
                        out=accT[:, :w], lhsT=wmsg_t[:], rhs=sT_sb[:, :w],
                        start=True, stop=False,
                    )
                    nc.tensor.matmul(
                        out=accT[:, :w], lhsT=rb_t[:],
                        rhs=cnt_sb[:, s0 * 128 : s0 * 128 + w],
                        start=False, stop=False,
                    )
                    nc.tensor.matmul(
                        out=accT[:, :w], lhsT=wself_t[:],
                        rhs=xT_sb[:, s0 * 128 : s0 * 128 + w],
                        start=False, stop=True,
                    )
                    nc.scalar.activation(
                        out=o14[:, s0 * 128 : s0 * 128 + w],
                        in_=accT[:, :w],
                        func=mybir.ActivationFunctionType.Relu,
                        bias=bcol_t[:, 0:1],
                    )
                nc.sync.dma_start(
                    out=out_d.ap()[:, g0 * 128 : (g0 + SB_BLOCKS) * 128],
                    in_=o14[:],
                )

    nc.compile()
    return nc


def _prep(inputs):
    """Host-side sharding/layout. Returns (in_maps, static_key, layout)."""
    x = np.ascontiguousarray(np.asarray(inputs["x"], dtype=np.float32))
    source = np.asarray(inputs["source"]).astype(np.int64)
    target = np.asarray(inputs["target"]).astype(np.int64)
    edge_type = np.asarray(inputs["edge_type"]).astype(np.int64)
    ew = np.asarray(inputs["edge_weights"], dtype=np.float32)
    w_msg = np.ascontiguousarray(np.asarray(inputs["W_msg"], dtype=np.float32))
    rel_bias = np.ascontiguousarray(np.asarray(inputs["rel_bias"], dtype=np.float32))
    w_self = np.ascontiguousarray(np.asarray(inputs["W_self"], dtype=np.float32))
    b = np.asarray(inputs["b"], dtype=np.float32).reshape(D, 1)

    n = x.shape[0]
    assert n == NUM_NODES

    xbf = x.astype(ml_dtypes.bfloat16)
    w_msg_b = w_msg.astype(ml_dtypes.bfloat16)
    w_self_b = w_self.astype(ml_dtypes.bfloat16)
    rb_b = rel_bias.astype(ml_dtypes.bfloat16)
    iota_t = np.broadcast_to(
        np.arange(128, dtype=np.float32), (128, 128)
    ).astype(ml_dtypes.bfloat16)
    iota_t = np.ascontiguousarray(iota_t)

    core = target // NODES_PER_CORE
    tgt_local = target - core * NODES_PER_CORE
    blk = tgt_local >> 7
    tgt_in_blk = tgt_local & 127
    subt = source // SUBT_ROWS
    src_local = source - subt * SUBT_ROWS

    # per (core, blk, subtable) edge index lists
    key = ((core * NBLK + blk) * N_SUBT + subt).astype(np.int64)
    order = np.argsort(key, kind="stable")
    key_s = key[order]
    uniq, starts = np.unique(key_s, return_index=True)
    counts = np.diff(np.append(starts, key_s.shape[0]))

    cnt = np.zeros((N_CORES, NBLK, N_SUBT), dtype=np.int64)
    ci = uniq // (NBLK * N_SUBT)
    bi = (uniq // N_SUBT) % NBLK
    ti = uniq % N_SUBT
    cnt[ci, bi, ti] = counts

    # static chunk capacity per (blk, subtable): max over cores
    c_bt = np.ceil(cnt.max(axis=0) / 128).astype(np.int64)  # (NBLK, N_SUBT)
    empty = c_bt.sum(axis=1) == 0
    c_bt[empty, 0] = 1

    nchunks_sbt = [
        [int(c_bt[sb * SB_BLOCKS : (sb + 1) * SB_BLOCKS, t].sum()) for t in range(N_SUBT)]
        for sb in range(N_SB)
    ]
    NC_TOT = int(c_bt.sum())

    # global chunk ids: order is (sb, t, blk-within-sb, chunk)
    gchunk_of = np.zeros((NBLK, N_SUBT), dtype=np.int64)  # first chunk id
    slot_of = np.zeros((NBLK, N_SUBT), dtype=np.int64)    # first slot in (sb,t) tile
    slotbase_sbt = [[0] * N_SUBT for _ in range(N_SB)]
    g = 0
    for sb in range(N_SB):
        for t in range(N_SUBT):
            slotbase_sbt[sb][t] = g
            s = 0
            for bi2 in range(SB_BLOCKS):
                bb = sb * SB_BLOCKS + bi2
                gchunk_of[bb, t] = g
                slot_of[bb, t] = s
                g += int(c_bt[bb, t])
                s += int(c_bt[bb, t])
    assert g == NC_TOT

    chunk_plan = []
    for bb in range(NBLK):
        plan = []
        for t in range(N_SUBT):
            for c in range(int(c_bt[bb, t])):
                plan.append((t, int(slot_of[bb, t] + c), int(gchunk_of[bb, t] + c)))
        chunk_plan.append(plan)

    n_idx_cols = NC_TOT * 8

    # offsets of edge groups in the sorted edge array, per (core, blk, subt)
    start_of = {}
    for u, s0, c0 in zip(uniq, starts, counts):
        start_of[int(u)] = (int(s0), int(c0))

    ew_bf = ew.astype(ml_dtypes.bfloat16)

    in_maps = []
    for c in range(N_CORES):
        gidx = np.zeros((128, n_idx_cols), dtype=np.int16)
        tgt_m = np.full((128, NC_TOT), 200.0, dtype=ml_dtypes.bfloat16)
        ew_m = np.zeros((128, NC_TOT), dtype=ml_dtypes.bfloat16)

        icol = 0
        for sb in range(N_SB):
            for t in range(N_SUBT):
                nck = nchunks_sbt[sb][t]
                if nck == 0:
                    continue
                nslots = nck * 128
                idxs = np.zeros(nslots, dtype=np.int16)
                for bi2 in range(SB_BLOCKS):
                    bb = sb * SB_BLOCKS + bi2
                    k = (c * NBLK + bb) * N_SUBT + t
                    s0, n_e = start_of.get(k, (0, 0))
                    sl0 = (int(slot_of[bb, t]) - int(slot_of[sb * SB_BLOCKS, t])) * 128
                    g0 = int(gchunk_of[bb, t])
                    if n_e:
                        eids = order[s0 : s0 + n_e]
                        idxs[sl0 : sl0 + n_e] = src_local[eids].astype(np.int16)
                        # meta: per chunk column = global chunk id
                        for cc in range(int(c_bt[bb, t])):
                            lo = cc * 128
                            hi = min(n_e, lo + 128)
                            if hi <= lo:
                                break
                            ecol = eids[lo:hi]
                            npart = hi - lo
                            parts = np.arange(npart)
                            tgt_m[parts, g0 + cc] = tgt_in_blk[ecol].astype(
                                ml_dtypes.bfloat16
                            )
                            ew_m[parts, g0 + cc] = ew_bf[ecol]
                # wrap idxs: element j -> partition j%16, col j//16; replicate x8
                wrapped = idxs.reshape(nslots // 16, 16).T  # (16, nslots/16)
                gidx[:, icol : icol + nslots // 16] = np.tile(wrapped, (8, 1))
                icol += nslots // 16
        assert icol == n_idx_cols

        lo = c * NODES_PER_CORE
        hi = min(lo + NODES_PER_CORE, NUM_NODES)
        xs = np.zeros((NODES_PER_CORE, D), dtype=np.float32)
        xs[: hi - lo] = x[lo:hi]
        xT = np.ascontiguousarray(xs.T.astype(ml_dtypes.bfloat16))

        # weighted relation counts per local node: cnt_w[r, j]
        emask = core == c
        r_e = edge_type[emask]
        j_e = tgt_local[emask]
        w_e = ew[emask]
        cw = np.zeros((NUM_REL, NODES_PER_CORE), dtype=np.float64)
        np.add.at(cw, (r_e, j_e), w_e)
        cw = cw.astype(ml_dtypes.bfloat16)

        in_maps.append(
            {
                "xbf": xbf,
                "xT_shard": xT,
                "w_msg_b": w_msg_b,
                "w_self_b": w_self_b,
                "rb_b": rb_b,
                "b_col": b,
                "iota_d": iota_t,
                "cnt_w": cw,
                "gidx": gidx,
                "tgt_meta": tgt_m,
                "ew_meta": ew_m,
            }
        )

    static_key = tuple(c_bt.flatten().tolist())
    return in_maps, static_key, (nchunks_sbt, chunk_plan, slotbase_sbt)


def kernel(**inputs) -> np.ndarray:
    from concourse import bass_utils

    in_maps, static_key, (nchunks_sbt, chunk_plan, slotbase_sbt) = _prep(inputs)

    nc = _kernel_cache.get(static_key)
    if nc is None:
        nc = _build_and_compile(static_key, nchunks_sbt, chunk_plan, slotbase_sbt)
        _kernel_cache[static_key] = nc

    res = bass_utils.run_bass_kernel_spmd(
        nc, in_maps, core_ids=list(range(N_CORES))
    )
    parts = [res.results[c]["out"].T for c in range(N_CORES)]
    full = np.concatenate(parts, axis=0)[:NUM_NODES]
    return np.ascontiguousarray(full, dtype=np.float32)
